# revision 14
# baseline (speedup 1.0000x reference)
"""FM layer (factorization machine) Trainium2 Bass kernel.

Computes, for x (B, N), W (1, N), b (1,), V (N, K):
    out = x @ W.T + b + 0.5*sum((x@V)**2, axis=1) - 0.5*||V.sum(0)||^2 * (x.sum(1))**2

Strategy: data-parallel over B across 8 NeuronCores (2048 rows/core).
The host ships x already in bf16 AND already transposed/tiled into the
exact SBUF layout the PE needs:

    X3[n, m, g, b] = x[128*m + b, 128*g + n]     (per core)

so each m-tile's DMA is one fully-contiguous per-partition run (8KB) and
the device does ZERO transposes / casts.  Per 128-row m-tile the PE
accumulates   y = x_tile @ [V | w | 1]   (128, 132) over the 32
contraction chunks (stationary = x chunk, FWL-eligible bf16; moving =
the shared M tile).  Epilogue on ACT+DVE:
    out = (y_w + b) + 0.5*sum_k y_k^2 - (sqrt(c/2)*y_ones)^2,  c=||V.sum(0)||^2

The kernel is DMA-roofline-bound (~17.9MB @ ~345 GB/s ~= 52us/core), so
ramp/tail are minimized: M and the first/last x chunks are split into
quarter-DMAs so the PE starts ~1.6us after the first DMA byte and
finishes ~1us after the last, with the DMA queue never idle in between.

Numerics: the only output-scale-critical quantity is xsum (the ones
column); bf16-rounded x gives ~1.5e-3 max rel err (tolerance 2e-2).

Hardcoded shapes: B=16384, N=4096, K=128, 8 cores.
"""

from contextlib import ExitStack

import numpy as np

import concourse.bass as bass
import concourse.mybir as mybir
import concourse.tile as tile
from concourse import bacc
from concourse.bass import ts
from concourse.bass_utils import run_bass_kernel_spmd
from concourse.masks import make_identity

N_CORES = 8
B_FULL = 16384
N_DIM = 4096
K_DIM = 128
B_SHARD = B_FULL // N_CORES   # 2048
M_TILES = B_SHARD // 128      # 16
G = N_DIM // 128              # 32 contraction chunks
GQ = G // 4                   # 8 chunks per quarter-DMA
NF = K_DIM + 2                # y columns: [V (128) | w (1) | ones (1)]
NF_PAD = 132
F32 = mybir.dt.float32
BF16 = mybir.dt.bfloat16
AF = mybir.ActivationFunctionType
ALU = mybir.AluOpType


def build_program(mode="full", repeats=1):
    """Trace + schedule + compile the per-core Bass program.

    mode: "full" | "dmaonly" (only x DMA) | "nomm" (skip epilogue).
    repeats: hardware-loop the whole body (timing deltas only).
    """
    nc = bacc.Bacc("TRN2", target_bir_lowering=False, debug=False)
    x_d = nc.dram_tensor("xt", [128, M_TILES * G * 128], BF16,
                         kind="ExternalInput").ap()
    m_d = nc.dram_tensor("mw", [128, G * NF_PAD], BF16,
                         kind="ExternalInput").ap()
    aux_d = nc.dram_tensor("aux", [128, 2], F32, kind="ExternalInput").ap()
    out_d = nc.dram_tensor("out", [B_SHARD, 1], F32, kind="ExternalOutput").ap()

    with tile.TileContext(nc) as tc, ExitStack() as ctx:
        const_pool = ctx.enter_context(tc.tile_pool(name="const", bufs=1))
        x_pool = ctx.enter_context(tc.tile_pool(name="xin", bufs=5))
        q_pool = ctx.enter_context(tc.tile_pool(name="xq", bufs=2))
        sc_pool = ctx.enter_context(tc.tile_pool(name="scratch", bufs=2))
        psy_pool = ctx.enter_context(tc.tile_pool(name="psy", bufs=4, space="PSUM"))
        pso_pool = ctx.enter_context(tc.tile_pool(name="pso", bufs=1, space="PSUM"))

        ident_f32 = const_pool.tile([128, 128], F32)
        make_identity(nc, ident_f32[:])

        # M = [V | w | 1] in 4 quarter tiles of 8 g-chunks each, so the
        # first matmul only waits for one quarter (~270KB), not 1.08MB.
        m_v = m_d.rearrange("p (g f) -> p g f", g=G)
        m_sb = [const_pool.tile([128, GQ, NF_PAD], BF16, name=f"msb{j}",
                                tag=f"msb{j}") for j in range(4)]

        # M quarters + aux ride the scalar HWDGE queue so their issue cost
        # overlaps the x-chunk issues on the sync queue.
        aux_sb = const_pool.tile([128, 2], F32)
        nc.scalar.dma_start(m_sb[0][:], m_v[:, ts(0, GQ)])
        # First x chunk in quarters too (~262KB each) on the sync queue.
        x0q = [q_pool.tile([128, GQ * 128], BF16, name=f"x0q{j}",
                            tag=f"x0q{j}") for j in range(4)]
        for j in range(4):
            nc.sync.dma_start(x0q[j][:], x_d[:, ts(j, GQ * 128)])
        for j in range(1, 4):
            nc.scalar.dma_start(m_sb[j][:], m_v[:, ts(j, GQ)])
        nc.scalar.dma_start(aux_sb[:], aux_d[:])

        out_stage = const_pool.tile([128, M_TILES], F32)

        def x_src(m, g0, ng):
            """HBM slice for g-chunks [g0, g0+ng) of m-tile m."""
            base = m * G * 128
            return x_d[:, base + g0 * 128 : base + (g0 + ng) * 128]

        def emit_mtile(m, parts):
            """parts: list of (tile, g0, ng) covering all 32 g-chunks."""
            if mode == "dmaonly":
                return
            psy = psy_pool.tile([128, NF_PAD], F32)
            for t, g0, ng in parts:
                for jj in range(ng):
                    g = g0 + jj
                    nc.tensor.matmul(
                        psy[:], lhsT=t[:, ts(jj, 128)], rhs=m_sb[g // GQ][:, g % GQ],
                        start=(g == 0), stop=(g == G - 1),
                    )
            if mode == "nomm":
                return
            # Epilogue:
            #   sq_acc = sum_k (x@V)_k^2
            #   t3     = (xsum * sqrt(c/2))^2 = 0.5*c*xsum^2
            #   u      = 0.5*sq_acc - t3
            #   out    = (lin + b) + u
            scr = sc_pool.tile([128, K_DIM], F32)
            sq_acc = sc_pool.tile([128, 1], F32)
            nc.scalar.activation(
                scr[:], psy[:, 0:K_DIM], AF.Square, accum_out=sq_acc[:]
            )
            t3 = sc_pool.tile([128, 1], F32)
            nc.scalar.activation(
                t3[:], psy[:, K_DIM + 1 : K_DIM + 2], AF.Square,
                scale=aux_sb[:, 1:2],
            )
            u = sc_pool.tile([128, 1], F32)
            nc.vector.scalar_tensor_tensor(
                out=u[:], in0=sq_acc[:], scalar=0.5, in1=t3[:],
                op0=ALU.mult, op1=ALU.subtract,
            )
            nc.vector.scalar_tensor_tensor(
                out=out_stage[:, m : m + 1], in0=psy[:, K_DIM : K_DIM + 1],
                scalar=aux_sb[:, 0:1], in1=u[:], op0=ALU.add, op1=ALU.add,
            )

        # Output staging: gather out_stage [128, 8] -> [8, 128] per half so
        # the final DMA writes contiguous 512B runs per partition.  The
        # first half ships mid-stream; only the second is on the tail.
        H = M_TILES // 2
        out_v = out_d.rearrange("(h m p) o -> h m (p o)", h=2, p=128)

        def emit_out_half(h):
            pso = pso_pool.tile([H, 128], F32, name=f"pso{h}", tag=f"pso{h}")
            nc.tensor.transpose(pso[:], out_stage[:, h * H : (h + 1) * H],
                                ident_f32[:])
            o_sb = sc_pool.tile([H, 128], F32, name=f"osb{h}", tag=f"osb{h}")
            nc.vector.tensor_copy(o_sb[:], pso[:])
            # scalar HWDGE queue: never blocks the x-chunk FIFO on sync
            nc.scalar.dma_start(out_v[h], o_sb[:])

        def emit_body():
            # m-tile 0 from the ramp quarters.
            emit_mtile(0, [(x0q[j], j * GQ, GQ) for j in range(4)])
            # m-tiles 1..11: one 1.05MB DMA each (peak HBM efficiency).
            for m in range(1, M_TILES - 4):
                xt = x_pool.tile([128, G * 128], BF16, name=f"xt{m}",
                                 tag="xf")
                nc.sync.dma_start(xt[:], x_src(m, 0, G))
                emit_mtile(m, [(xt, 0, G)])
                if m == H - 1:
                    emit_out_half(0)
            # last four m-tiles in quarters: each gives the PE ~0.4us of
            # catch-up so it trails the stream by ~1 quarter at the end.
            for mL in range(M_TILES - 4, M_TILES - 1):
                xLq = [q_pool.tile([128, GQ * 128], BF16, name=f"xq{mL}_{j}",
                                    tag=f"xq{j}") for j in range(4)]
                for j in range(4):
                    nc.sync.dma_start(xLq[j][:], x_src(mL, j * GQ, GQ))
                emit_mtile(mL, [(xLq[j], j * GQ, GQ) for j in range(4)])
            # very last m-tile: 3 quarters + 2 eighths so only 4 matmuls
            # remain after the final DMA byte lands.
            mZ = M_TILES - 1
            GE = GQ // 2
            xZ = [q_pool.tile([128, GQ * 128], BF16, name=f"xz{j}",
                               tag=f"xq{j}") for j in range(3)]
            for j in range(3):
                nc.sync.dma_start(xZ[j][:], x_src(mZ, j * GQ, GQ))
            xE = [q_pool.tile([128, GE * 128], BF16, name=f"xe{j}",
                               tag=f"xe{j}") for j in range(2)]
            for j in range(2):
                nc.sync.dma_start(xE[j][:], x_src(mZ, 3 * GQ + j * GE, GE))
            emit_mtile(mZ, [(xZ[j], j * GQ, GQ) for j in range(3)]
                           + [(xE[j], 3 * GQ + j * GE, GE) for j in range(2)])
            emit_out_half(1)

        if repeats == 1:
            emit_body()
        else:
            with tc.For_i(0, repeats, 1):
                emit_body()


    nc.compile()
    return nc


def host_prep(x, W, b, V):
    """Per-core input maps: x bf16, transposed+tiled; tiny tensors replicated."""
    import ml_dtypes

    bf = ml_dtypes.bfloat16
    x = np.ascontiguousarray(x, dtype=np.float32)
    W = np.asarray(W, dtype=np.float32)
    b = np.asarray(b, dtype=np.float32)
    V = np.asarray(V, dtype=np.float32)

    # X3[core][n, m, g, b] = x[core*2048 + 128m + b, 128g + n], bf16.
    xb = x.astype(bf)
    X3 = xb.reshape(N_CORES, M_TILES, 128, G, 128).transpose(0, 4, 1, 3, 2)
    X3 = np.ascontiguousarray(X3).reshape(N_CORES, 128, M_TILES * G * 128)

    M = np.zeros((N_DIM, NF_PAD), dtype=np.float32)
    M[:, :K_DIM] = V
    M[:, K_DIM] = W[0]
    M[:, K_DIM + 1] = 1.0
    # M2[p, g, f] = M[128g + p, f], bf16, per-partition contiguous.
    M2 = np.ascontiguousarray(
        M.astype(bf).reshape(G, 128, NF_PAD).transpose(1, 0, 2)
    ).reshape(128, G * NF_PAD)

    s = V.astype(np.float64).sum(axis=0)
    c = float(s @ s)
    aux = np.zeros((128, 2), dtype=np.float32)
    aux[:, 0] = b[0]
    aux[:, 1] = np.sqrt(0.5 * c)

    return [{"xt": X3[core], "mw": M2, "aux": aux} for core in range(N_CORES)]


_prog_cache = {}


def _get_program(mode="full", repeats=1):
    key = (mode, repeats)
    if key not in _prog_cache:
        _prog_cache[key] = build_program(mode=mode, repeats=repeats)
    return _prog_cache[key]


def run(x, W, b, V, trace=False, retries=4, mode="full", **kw):
    nc = _get_program(mode=mode)
    in_maps = host_prep(x, W, b, V)
    last_exc = None
    for attempt in range(retries):
        try:
            res = run_bass_kernel_spmd(nc, in_maps, core_ids=list(range(N_CORES)),
                                       trace=trace, **kw)
            break
        except Exception as e:  # transient NRT_EXEC_UNIT flakes observed
            last_exc = e
            import time as _time

            print(f"kernel attempt {attempt} failed ({type(e).__name__}); retrying")
            _time.sleep(2.0)
    else:
        raise last_exc
    out = np.concatenate([r["out"] for r in res.results], axis=0)
    return out, res


def kernel(x, W, b, V):
    out, _ = run(x, W, b, V)
    return out


# revision 15
# speedup vs baseline: 1.1031x; 1.1031x over previous
"""FM layer (factorization machine) Trainium2 Bass kernel.

Computes, for x (B, N), W (1, N), b (1,), V (N, K):
    out = x @ W.T + b + 0.5*sum((x@V)**2, axis=1) - 0.5*||V.sum(0)||^2 * (x.sum(1))**2

Strategy: data-parallel over B across 8 NeuronCores (2048 rows/core).
The host ships x already in bf16 AND already transposed/tiled into the
exact SBUF layout the PE needs:

    X3[n, m, g, b] = x[128*m + b, 128*g + n]     (per core)

so each m-tile's DMA is one fully-contiguous per-partition run (8KB) and
the device does ZERO transposes / casts.  Per 128-row m-tile the PE
accumulates   y = x_tile @ [V | w | 1]   (128, 132) over the 32
contraction chunks (stationary = x chunk, FWL-eligible bf16; moving =
the shared M tile).  Epilogue on ACT+DVE:
    out = (y_w + b) + 0.5*sum_k y_k^2 - (sqrt(c/2)*y_ones)^2,  c=||V.sum(0)||^2

The kernel is DMA-roofline-bound (~17.9MB @ ~345 GB/s ~= 52us/core), so
ramp/tail are minimized: M and the first/last x chunks are split into
quarter-DMAs so the PE starts ~1.6us after the first DMA byte and
finishes ~1us after the last, with the DMA queue never idle in between.

Numerics: the only output-scale-critical quantity is xsum (the ones
column); bf16-rounded x gives ~1.5e-3 max rel err (tolerance 2e-2).

Hardcoded shapes: B=16384, N=4096, K=128, 8 cores.
"""

from contextlib import ExitStack

import numpy as np

import concourse.bass as bass
import concourse.mybir as mybir
import concourse.tile as tile
from concourse import bacc
from concourse.bass import ts
from concourse.bass_utils import run_bass_kernel_spmd
from concourse.masks import make_identity

N_CORES = 8
B_FULL = 16384
N_DIM = 4096
K_DIM = 128
B_SHARD = B_FULL // N_CORES   # 2048
M_TILES = B_SHARD // 128      # 16
G = N_DIM // 128              # 32 contraction chunks
GQ = G // 4                   # 8 chunks per quarter-DMA
NF = K_DIM + 2                # y columns: [V (128) | w (1) | ones (1)]
NF_PAD = 132
F32 = mybir.dt.float32
BF16 = mybir.dt.bfloat16
AF = mybir.ActivationFunctionType
ALU = mybir.AluOpType


def build_program(mode="full", repeats=1):
    """Trace + schedule + compile the per-core Bass program.

    mode: "full" | "dmaonly" (only x DMA) | "nomm" (skip epilogue).
    repeats: hardware-loop the whole body (timing deltas only).
    """
    nc = bacc.Bacc("TRN2", target_bir_lowering=False, debug=False)
    x_d = nc.dram_tensor("xt", [128, M_TILES * G * 128], BF16,
                         kind="ExternalInput").ap()
    m_d = nc.dram_tensor("mw", [128, G * NF_PAD], BF16,
                         kind="ExternalInput").ap()
    aux_d = nc.dram_tensor("aux", [128, 2], F32, kind="ExternalInput").ap()
    out_d = nc.dram_tensor("out", [B_SHARD, 1], F32, kind="ExternalOutput").ap()

    with tile.TileContext(nc) as tc, ExitStack() as ctx:
        const_pool = ctx.enter_context(tc.tile_pool(name="const", bufs=1))
        x_pool = ctx.enter_context(tc.tile_pool(name="xin", bufs=5))
        q_pool = ctx.enter_context(tc.tile_pool(name="xq", bufs=2))
        sc_pool = ctx.enter_context(tc.tile_pool(name="scratch", bufs=2))
        psy_pool = ctx.enter_context(tc.tile_pool(name="psy", bufs=4, space="PSUM"))
        pso_pool = ctx.enter_context(tc.tile_pool(name="pso", bufs=1, space="PSUM"))

        ident_f32 = const_pool.tile([128, 128], F32)
        make_identity(nc, ident_f32[:])

        # M = [V | w | 1] in 4 quarter tiles of 8 g-chunks each, so the
        # first matmul only waits for one quarter (~270KB), not 1.08MB.
        m_v = m_d.rearrange("p (g f) -> p g f", g=G)
        m_sb = [const_pool.tile([128, GQ, NF_PAD], BF16, name=f"msb{j}",
                                tag=f"msb{j}") for j in range(4)]

        # M quarters + aux ride the scalar HWDGE queue so their issue cost
        # overlaps the x-chunk issues on the sync queue.
        aux_sb = const_pool.tile([128, 2], F32)
        nc.scalar.dma_start(m_sb[0][:], m_v[:, ts(0, GQ)])
        # First x chunk in quarters too (~262KB each) on the sync queue.
        x0q = [q_pool.tile([128, GQ * 128], BF16, name=f"x0q{j}",
                            tag=f"x0q{j}") for j in range(4)]
        for j in range(4):
            nc.sync.dma_start(x0q[j][:], x_d[:, ts(j, GQ * 128)])
        for j in range(1, 4):
            nc.scalar.dma_start(m_sb[j][:], m_v[:, ts(j, GQ)])
        nc.scalar.dma_start(aux_sb[:], aux_d[:])

        out_stage = const_pool.tile([128, M_TILES], F32)

        def x_src(m, g0, ng):
            """HBM slice for g-chunks [g0, g0+ng) of m-tile m."""
            base = m * G * 128
            return x_d[:, base + g0 * 128 : base + (g0 + ng) * 128]

        def emit_mtile(m, parts):
            """parts: list of (tile, g0, ng) covering all 32 g-chunks."""
            if mode == "dmaonly":
                return
            psy = psy_pool.tile([128, NF_PAD], F32)
            for t, g0, ng in parts:
                for jj in range(ng):
                    g = g0 + jj
                    nc.tensor.matmul(
                        psy[:], lhsT=t[:, ts(jj, 128)], rhs=m_sb[g // GQ][:, g % GQ],
                        start=(g == 0), stop=(g == G - 1),
                    )
            if mode == "nomm":
                return
            # Epilogue:
            #   sq_acc = sum_k (x@V)_k^2
            #   t3     = (xsum * sqrt(c/2))^2 = 0.5*c*xsum^2
            #   u      = 0.5*sq_acc - t3
            #   out    = (lin + b) + u
            scr = sc_pool.tile([128, K_DIM], F32)
            sq_acc = sc_pool.tile([128, 1], F32)
            nc.scalar.activation(
                scr[:], psy[:, 0:K_DIM], AF.Square, accum_out=sq_acc[:]
            )
            t3 = sc_pool.tile([128, 1], F32)
            nc.scalar.activation(
                t3[:], psy[:, K_DIM + 1 : K_DIM + 2], AF.Square,
                scale=aux_sb[:, 1:2],
            )
            u = sc_pool.tile([128, 1], F32)
            nc.vector.scalar_tensor_tensor(
                out=u[:], in0=sq_acc[:], scalar=0.5, in1=t3[:],
                op0=ALU.mult, op1=ALU.subtract,
            )
            nc.vector.scalar_tensor_tensor(
                out=out_stage[:, m : m + 1], in0=psy[:, K_DIM : K_DIM + 1],
                scalar=aux_sb[:, 0:1], in1=u[:], op0=ALU.add, op1=ALU.add,
            )

        # Output staging: gather out_stage [128, 8] -> [8, 128] per half so
        # the final DMA writes contiguous 512B runs per partition.  The
        # first half ships mid-stream; only the second is on the tail.
        H = M_TILES // 2
        out_v = out_d.rearrange("(h m p) o -> h m (p o)", h=2, p=128)

        def emit_out_half(h):
            pso = pso_pool.tile([H, 128], F32, name=f"pso{h}", tag=f"pso{h}")
            nc.tensor.transpose(pso[:], out_stage[:, h * H : (h + 1) * H],
                                ident_f32[:])
            o_sb = sc_pool.tile([H, 128], F32, name=f"osb{h}", tag=f"osb{h}")
            nc.vector.tensor_copy(o_sb[:], pso[:])
            # scalar HWDGE queue: never blocks the x-chunk FIFO on sync
            nc.scalar.dma_start(out_v[h], o_sb[:])

        def emit_body():
            # m-tile 0 from the ramp quarters.
            emit_mtile(0, [(x0q[j], j * GQ, GQ) for j in range(4)])
            # m-tiles 1..14: one 1.05MB DMA each -- full chunks sustain
            # ~415 GB/s; smaller pieces mid-stream drop to ~200 GB/s.
            for m in range(1, M_TILES - 1):
                xt = x_pool.tile([128, G * 128], BF16, name=f"xt{m}",
                                 tag="xf")
                nc.sync.dma_start(xt[:], x_src(m, 0, G))
                emit_mtile(m, [(xt, 0, G)])
                if m == H - 1:
                    emit_out_half(0)
            # very last m-tile: 3 quarters + 2 eighths (fresh buffers, so
            # the issue queue stays deep) -- only 4 matmuls remain after
            # the final DMA byte lands.
            mZ = M_TILES - 1
            GE = GQ // 2
            xZ = [q_pool.tile([128, GQ * 128], BF16, name=f"xz{j}",
                               tag=f"xq{j}") for j in range(3)]
            for j in range(3):
                nc.sync.dma_start(xZ[j][:], x_src(mZ, j * GQ, GQ))
            xE = [q_pool.tile([128, GE * 128], BF16, name=f"xe{j}",
                               tag=f"xe{j}") for j in range(2)]
            for j in range(2):
                nc.sync.dma_start(xE[j][:], x_src(mZ, 3 * GQ + j * GE, GE))
            emit_mtile(mZ, [(xZ[j], j * GQ, GQ) for j in range(3)]
                           + [(xE[j], 3 * GQ + j * GE, GE) for j in range(2)])
            emit_out_half(1)

        if repeats == 1:
            emit_body()
        else:
            with tc.For_i(0, repeats, 1):
                emit_body()


    nc.compile()
    return nc


def host_prep(x, W, b, V):
    """Per-core input maps: x bf16, transposed+tiled; tiny tensors replicated."""
    import ml_dtypes

    bf = ml_dtypes.bfloat16
    x = np.ascontiguousarray(x, dtype=np.float32)
    W = np.asarray(W, dtype=np.float32)
    b = np.asarray(b, dtype=np.float32)
    V = np.asarray(V, dtype=np.float32)

    # X3[core][n, m, g, b] = x[core*2048 + 128m + b, 128g + n], bf16.
    xb = x.astype(bf)
    X3 = xb.reshape(N_CORES, M_TILES, 128, G, 128).transpose(0, 4, 1, 3, 2)
    X3 = np.ascontiguousarray(X3).reshape(N_CORES, 128, M_TILES * G * 128)

    M = np.zeros((N_DIM, NF_PAD), dtype=np.float32)
    M[:, :K_DIM] = V
    M[:, K_DIM] = W[0]
    M[:, K_DIM + 1] = 1.0
    # M2[p, g, f] = M[128g + p, f], bf16, per-partition contiguous.
    M2 = np.ascontiguousarray(
        M.astype(bf).reshape(G, 128, NF_PAD).transpose(1, 0, 2)
    ).reshape(128, G * NF_PAD)

    s = V.astype(np.float64).sum(axis=0)
    c = float(s @ s)
    aux = np.zeros((128, 2), dtype=np.float32)
    aux[:, 0] = b[0]
    aux[:, 1] = np.sqrt(0.5 * c)

    return [{"xt": X3[core], "mw": M2, "aux": aux} for core in range(N_CORES)]


_prog_cache = {}


def _get_program(mode="full", repeats=1):
    key = (mode, repeats)
    if key not in _prog_cache:
        _prog_cache[key] = build_program(mode=mode, repeats=repeats)
    return _prog_cache[key]


def run(x, W, b, V, trace=False, retries=4, mode="full", **kw):
    nc = _get_program(mode=mode)
    in_maps = host_prep(x, W, b, V)
    last_exc = None
    for attempt in range(retries):
        try:
            res = run_bass_kernel_spmd(nc, in_maps, core_ids=list(range(N_CORES)),
                                       trace=trace, **kw)
            break
        except Exception as e:  # transient NRT_EXEC_UNIT flakes observed
            last_exc = e
            import time as _time

            print(f"kernel attempt {attempt} failed ({type(e).__name__}); retrying")
            _time.sleep(2.0)
    else:
        raise last_exc
    out = np.concatenate([r["out"] for r in res.results], axis=0)
    return out, res


def kernel(x, W, b, V):
    out, _ = run(x, W, b, V)
    return out
